# revision 35
# baseline (speedup 1.0000x reference)
"""Trainium2 kernel for DigitConvolutionalModel.

Model: x(B,784) -> reshape(28,28) -> conv3x3 'VALID' (cross-correlation)
       -> flatten(676) -> Linear(676,256)+ReLU -> Linear(256,10).

The conv is linear, so it folds into the first Linear:
    feat = x @ Wc          (Wc: 784x676 sparse conv matrix)
    h    = relu(feat @ w1 + b1) = relu(x @ (Wc @ w1) + b1)
Device work is then two GEMMs per batch tile:
    H^T = relu(W_eff^T-tiles . x^T + b1);  out^T = w2^T . H + b2

Sharding: pure data parallel over 8 cores (8192 rows each). The host
pre-transposes each shard to x^T (contraction dim on SBUF partitions) and
casts to bf16 so the PE streams it directly; weights are replicated.

Schedule notes (v3):
  - x DMAs on the sync ring (not saturated at 1024-col tiles); weights
    ride the scalar ring ahead of everything; output slabs keep their own
    gpsimd ring (a store queued behind x prefetch loads trips a FIFO
    slot-wait deadlock).  Only sync/scalar/gpsimd can initiate DMAs.
  - The PE warms up on dummy matmuls over a memset tile while chunk 0's
    x DMAs are in flight, so the p-state ramp happens before real work.
  - GEMM2 for chunk c is issued right after chunk c+1's first relu, so
    the PE never waits on the ACT that produces its input.
"""

import os
from contextlib import ExitStack

import numpy as np
import ml_dtypes

import concourse.bass as bass
import concourse.tile as tile
from concourse import bacc, mybir
from concourse.bass_utils import run_bass_kernel_spmd

N_CORES = 8
B = 65536
B_SHARD = B // N_CORES  # 8192
K = 784                 # contraction dim (pixels)
KT = 112                # k-tile partition size (7 * 112 = 784)
NKT = K // KT
CH = 256                # hidden channels
MT = 128                # m-tile (output channels per matmul)
NMT = CH // MT
OUT_CH = 10
OUT_PAD = 16            # padded output channels
SUB = 512               # matmul moving free dim / PSUM bank
OGRP = 2048             # output store granularity (batch columns)
N_WARM = 10             # dummy matmuls to ramp the PE p-state
BF16 = mybir.dt.bfloat16
F32 = mybir.dt.float32

_CACHE: dict = {}


def _build(b_shard: int):
    nc = bacc.Bacc(
        "TRN2",
        target_bir_lowering=False,
        debug=False,
        num_devices=N_CORES,
    )
    xT = nc.dram_tensor("xT", [K, b_shard], BF16, kind="ExternalInput")
    # GEMM1 weight tiles packed side by side, m-major: [112, (m*NKT+t)*MT + j]
    wta = nc.dram_tensor("wta", [KT, NKT * NMT * MT], BF16, kind="ExternalInput")
    b1a = nc.dram_tensor("b1a", [MT, NMT], F32, kind="ExternalInput")
    w2a = nc.dram_tensor("w2a", [MT, NMT * OUT_PAD], BF16, kind="ExternalInput")
    b2c = nc.dram_tensor("b2c", [OUT_PAD, 1], F32, kind="ExternalInput")
    outT = nc.dram_tensor("outT", [OUT_PAD, b_shard], F32, kind="ExternalOutput")

    relu = mybir.ActivationFunctionType.Relu
    # Light head chunk (its 7 x tiles are spread over all 3 DMA rings so
    # the first PSUM group is ready fast) and light tail chunk (short
    # final GEMM2 drain).
    chunks = [512] + [1024] * ((b_shard - 1024) // 1024) + [512]
    assert sum(chunks) == b_shard
    n_ogrp = b_shard // OGRP

    def blocks_of(csz):
        out, off = [], 0
        while off < csz:
            ln = min(SUB, csz - off)
            out.append((off, ln))
            off += ln
        return out

    with tile.TileContext(nc) as tc, ExitStack() as ctx:
        const = ctx.enter_context(tc.tile_pool(name="const", bufs=1))
        # GEMM2 constants in their own pool: sharing the bufs=1 const pool
        # with the GEMM1 weights trips a scheduler slot-wait deadlock.
        const2 = ctx.enter_context(tc.tile_pool(name="const2", bufs=1))
        opool = ctx.enter_context(tc.tile_pool(name="out", bufs=1))
        xpool = ctx.enter_context(tc.tile_pool(name="xin", bufs=4))
        hpool = ctx.enter_context(tc.tile_pool(name="h", bufs=4))
        hps = ctx.enter_context(
            tc.tile_pool(name="hps", bufs=2, space=bass.MemorySpace.PSUM)
        )
        ops = ctx.enter_context(
            tc.tile_pool(name="ops", bufs=3, space=bass.MemorySpace.PSUM)
        )

        # --- resident weights/biases.  GEMM1 weights alone on the scalar
        # ring (they gate the first real matmul); the small bias/GEMM2
        # constants ride the sync ring behind chunk 0's even x tiles. ---
        HW = NKT * MT
        wt_m = [
            const.tile([KT, HW], BF16, tag=f"wta{m}", name=f"wt_m{m}")
            for m in range(NMT)
        ]
        for m in range(NMT):
            nc.scalar.dma_start(wt_m[m][:], wta[:, m * HW:(m + 1) * HW])
        b1_all = const.tile([MT, NMT], F32, tag="b1a")
        w2_all = const2.tile([MT, NMT * OUT_PAD], BF16, tag="w2a")
        b2_sb = const2.tile([OUT_PAD, 1], F32, tag="b2")

        def w_sb(t, m):
            return wt_m[m][:, t * MT:(t + 1) * MT]

        # PE p-state warmup: dummy matmuls over a memset tile, no DMA deps,
        # so the PE ramps to full clock while chunk 0 is still in flight.
        warm = const2.tile([KT, 256], BF16, tag="warm")
        nc.vector.memset(warm[:], 0)
        wps = hps.tile([MT, 256], F32, tag="warm_ps", bufs=1)
        for _ in range(N_WARM):
            nc.tensor.matmul(wps[:], warm[:, :MT], warm[:], start=True, stop=True)

        # Output accumulates in SBUF, streamed out in OGRP slabs on the
        # GpSimd (SWDGE) ring so stores overlap compute and never queue
        # behind x prefetch loads (FIFO slot-wait deadlock).
        oall = [
            opool.tile([OUT_PAD, OGRP], F32, tag=f"o{g}", name=f"oall{g}")
            for g in range(n_ogrp)
        ]

        # GEMM2 runs one chunk behind GEMM1 (software pipeline): by the
        # time it streams h, the relu that produced h is long done, so the
        # PE never stalls on the ACT semaphore.
        pending = []  # [(hb, j0, slen)] sub-blocks awaiting GEMM2

        def flush_gemm2():
            while pending:
                hb2, j0, slen = pending.pop(0)
                po = ops.tile([OUT_PAD, slen], F32, tag="po", name="po")
                for m in range(NMT):
                    nc.tensor.matmul(
                        po[:],
                        w2_all[:, m * OUT_PAD:(m + 1) * OUT_PAD],
                        hb2[m][:],
                        start=(m == 0),
                        stop=(m == NMT - 1),
                    )
                g = j0 // OGRP
                nc.vector.tensor_scalar_add(
                    oall[g][:, j0 - g * OGRP:j0 - g * OGRP + slen],
                    po[:], b2_sb[:],
                )
                if (j0 + slen) % OGRP == 0:
                    nc.gpsimd.dma_start(
                        outT[:, g * OGRP:(g + 1) * OGRP], oall[g][:]
                    )

        # --- main loop over batch chunks ---
        coff = 0
        for c, csz in enumerate(chunks):
            xt = []
            for t in range(NKT):
                xtile = xpool.tile([KT, csz], BF16, tag=f"x{t}")
                # chunk 0's odd k-tiles ride the scalar ring (behind the
                # weights) so the first PSUM group is ready sooner.  Later
                # chunks must NOT: scalar-ring transfers queue ahead of the
                # ACTIVATEs and stall the PSUM-bank recycle (measured).
                eng = nc.scalar if (c == 0 and t % 2 == 1) else nc.sync
                eng.dma_start(
                    xtile[:], xT[t * KT:(t + 1) * KT, coff:coff + csz]
                )
                xt.append(xtile)
            if c == 0:
                # small one-time constants, behind chunk 0's x tiles
                nc.scalar.dma_start(b1_all[:], b1a[:, :])
                nc.scalar.dma_start(w2_all[:], w2a[:, :])
                nc.scalar.dma_start(b2_sb[:], b2c[:, :])
            # t-major matmul order: each x tile feeds all 2x2 (s, m) PSUM
            # accumulation groups as soon as it lands, so per-tile PE work
            # (~4 matmuls) exceeds per-tile DMA time and the head of the
            # run never starves waiting for a chunk's last k-tile.
            blocks = blocks_of(csz)
            pss = {
                (s, m): hps.tile([MT, SUB], F32, tag=f"ps{m}",
                                 name=f"ps_c{c}_s{s}_m{m}")
                for s in range(len(blocks))
                for m in range(NMT)
            }
            for t in range(NKT):
                for s, (soff, slen) in enumerate(blocks):
                    for m in range(NMT):
                        nc.tensor.matmul(
                            pss[s, m][:, :slen],
                            w_sb(t, m),
                            xt[t][:, soff:soff + slen],
                            start=(t == 0),
                            stop=(t == NKT - 1),
                        )
                if t == 3:
                    flush_gemm2()  # previous chunk's GEMM2, relus long done
            for s, (soff, slen) in enumerate(blocks):
                hb = []
                for m in range(NMT):
                    h = hpool.tile([MT, slen], BF16, tag=f"h{m}")
                    nc.scalar.activation(h[:], pss[s, m][:, :slen], relu,
                                         bias=b1_all[:, m:m + 1])
                    hb.append(h)
                pending.append((hb, coff + soff, slen))
            coff += csz
        flush_gemm2()

    nc.compile()
    return nc


def _get_nc(b_shard: int = B_SHARD):
    if b_shard not in _CACHE:
        _CACHE[b_shard] = _build(b_shard)
    return _CACHE[b_shard]


def _host_prep(x, w_conv, w1, b1, w2, b2, b_shard=B_SHARD):
    """Fold conv into w1, pack weights, and lay out per-core inputs."""
    bf16 = ml_dtypes.bfloat16
    # Conv matrix Wc[784, 676]: feat[:, oi*26+oj] = sum_{di,dj} x[:, (oi+di)*28+(oj+dj)] * w_conv[di,dj]
    w_conv = np.asarray(w_conv, np.float64)
    oi = np.arange(26)
    oj = np.arange(26)
    wc = np.zeros((784, 676), np.float64)
    for di in range(3):
        for dj in range(3):
            src = ((oi[:, None] + di) * 28 + (oj[None, :] + dj)).ravel()
            dst = (oi[:, None] * 26 + oj[None, :]).ravel()
            wc[src, dst] += w_conv[di, dj]
    w_eff = (wc @ np.asarray(w1, np.float64)).astype(bf16)  # [784, 256]

    # wta[p, (m*NKT+t)*MT + j] = w_eff[t*KT+p, m*MT+j]  (m-major)
    wta = np.ascontiguousarray(
        w_eff.reshape(NKT, KT, NMT, MT).transpose(1, 2, 0, 3).reshape(KT, -1)
    )
    # b1a[p, m] = b1[m*MT+p]
    b1a = np.ascontiguousarray(
        np.asarray(b1, np.float32).reshape(NMT, MT).T
    )
    # w2a[p, m*OUT_PAD + j] = w2_padded[m*MT+p, j]
    w2p = np.zeros((CH, OUT_PAD), bf16)
    w2p[:, :OUT_CH] = np.asarray(w2).astype(bf16)
    w2a = np.ascontiguousarray(
        w2p.reshape(NMT, MT, OUT_PAD).transpose(1, 0, 2).reshape(MT, -1)
    )
    b2c = np.zeros((OUT_PAD, 1), np.float32)
    b2c[:OUT_CH, 0] = np.asarray(b2, np.float32)

    x_bf = np.asarray(x).astype(bf16)  # [B, 784]
    in_maps = []
    for c in range(N_CORES):
        shard = x_bf[c * b_shard:(c + 1) * b_shard]
        in_maps.append(
            {
                "xT": np.ascontiguousarray(shard.T),  # [784, b_shard]
                "wta": wta,
                "b1a": b1a,
                "w2a": w2a,
                "b2c": b2c,
            }
        )
    return in_maps


LAST_RESULT = None  # BassKernelResults of the most recent run (for test harness)


def kernel(x, w_conv, w1, b1, w2, b2):
    global LAST_RESULT
    nc = _get_nc()
    in_maps = _host_prep(x, w_conv, w1, b1, w2, b2)
    trace = bool(int(os.environ.get("KERNEL_TRACE", "0")))
    res = run_bass_kernel_spmd(
        nc, in_maps, list(range(N_CORES)), trace=trace,
        tmpdir=os.environ.get("KERNEL_TMPDIR") or None,
    )
    LAST_RESULT = res
    out = np.empty((B, OUT_CH), np.float32)
    for c in range(N_CORES):
        out[c * B_SHARD:(c + 1) * B_SHARD] = res.results[c]["outT"][:OUT_CH].T
    return out


# revision 36
# speedup vs baseline: 1.0365x; 1.0365x over previous
"""Trainium2 kernel for DigitConvolutionalModel.

Model: x(B,784) -> reshape(28,28) -> conv3x3 'VALID' (cross-correlation)
       -> flatten(676) -> Linear(676,256)+ReLU -> Linear(256,10).

The conv is linear, so it folds into the first Linear:
    feat = x @ Wc          (Wc: 784x676 sparse conv matrix)
    h    = relu(feat @ w1 + b1) = relu(x @ (Wc @ w1) + b1)
Device work is then two GEMMs per batch tile:
    H^T = relu(W_eff^T-tiles . x^T + b1);  out^T = w2^T . H + b2

Sharding: pure data parallel over 8 cores (8192 rows each). The host
pre-transposes each shard to x^T (contraction dim on SBUF partitions) and
casts to bf16 so the PE streams it directly; weights are replicated.

Schedule notes (v3):
  - x DMAs on the sync ring (not saturated at 1024-col tiles); weights
    ride the scalar ring ahead of everything; output slabs keep their own
    gpsimd ring (a store queued behind x prefetch loads trips a FIFO
    slot-wait deadlock).  Only sync/scalar/gpsimd can initiate DMAs.
  - The PE warms up on dummy matmuls over a memset tile while chunk 0's
    x DMAs are in flight, so the p-state ramp happens before real work.
  - GEMM2 for chunk c is issued right after chunk c+1's first relu, so
    the PE never waits on the ACT that produces its input.
"""

import os
from contextlib import ExitStack

import numpy as np
import ml_dtypes

import concourse.bass as bass
import concourse.tile as tile
from concourse import bacc, mybir
from concourse.bass_utils import run_bass_kernel_spmd

N_CORES = 8
B = 65536
B_SHARD = B // N_CORES  # 8192
K = 784                 # contraction dim (pixels)
KT = 112                # k-tile partition size (7 * 112 = 784)
NKT = K // KT
CH = 256                # hidden channels
MT = 128                # m-tile (output channels per matmul)
NMT = CH // MT
OUT_CH = 10
OUT_PAD = 16            # padded output channels
SUB = 512               # matmul moving free dim / PSUM bank
OGRP = 2048             # output store granularity (batch columns)
N_WARM = 10             # dummy matmuls to ramp the PE p-state
BF16 = mybir.dt.bfloat16
F32 = mybir.dt.float32

_CACHE: dict = {}


def _build(b_shard: int):
    nc = bacc.Bacc(
        "TRN2",
        target_bir_lowering=False,
        debug=False,
        num_devices=N_CORES,
    )
    xT = nc.dram_tensor("xT", [K, b_shard], BF16, kind="ExternalInput")
    # GEMM1 weight tiles packed side by side, m-major: [112, (m*NKT+t)*MT + j]
    wta = nc.dram_tensor("wta", [KT, NKT * NMT * MT], BF16, kind="ExternalInput")
    b1a = nc.dram_tensor("b1a", [MT, NMT], F32, kind="ExternalInput")
    w2a = nc.dram_tensor("w2a", [MT, NMT * OUT_PAD], BF16, kind="ExternalInput")
    b2c = nc.dram_tensor("b2c", [OUT_PAD, 1], F32, kind="ExternalInput")
    outT = nc.dram_tensor("outT", [OUT_PAD, b_shard], F32, kind="ExternalOutput")

    relu = mybir.ActivationFunctionType.Relu
    # Light head chunk (its 7 x tiles are spread over all 3 DMA rings so
    # the first PSUM group is ready fast) and light tail chunk (short
    # final GEMM2 drain).
    chunks = [512] + [1024] * ((b_shard - 1024) // 1024) + [512]
    assert sum(chunks) == b_shard
    n_ogrp = b_shard // OGRP

    def blocks_of(csz):
        out, off = [], 0
        while off < csz:
            ln = min(SUB, csz - off)
            out.append((off, ln))
            off += ln
        return out

    with tile.TileContext(nc) as tc, ExitStack() as ctx:
        const = ctx.enter_context(tc.tile_pool(name="const", bufs=1))
        # GEMM2 constants in their own pool: sharing the bufs=1 const pool
        # with the GEMM1 weights trips a scheduler slot-wait deadlock.
        const2 = ctx.enter_context(tc.tile_pool(name="const2", bufs=1))
        opool = ctx.enter_context(tc.tile_pool(name="out", bufs=1))
        xpool = ctx.enter_context(tc.tile_pool(name="xin", bufs=4))
        hpool = ctx.enter_context(tc.tile_pool(name="h", bufs=4))
        hps = ctx.enter_context(
            tc.tile_pool(name="hps", bufs=2, space=bass.MemorySpace.PSUM)
        )
        ops = ctx.enter_context(
            tc.tile_pool(name="ops", bufs=3, space=bass.MemorySpace.PSUM)
        )

        # --- resident weights/biases.  GEMM1 weights alone on the scalar
        # ring (they gate the first real matmul); the small bias/GEMM2
        # constants ride the sync ring behind chunk 0's even x tiles. ---
        HW = NKT * MT
        wt_m = [
            const.tile([KT, HW], BF16, tag=f"wta{m}", name=f"wt_m{m}")
            for m in range(NMT)
        ]
        for m in range(NMT):
            nc.scalar.dma_start(wt_m[m][:], wta[:, m * HW:(m + 1) * HW])
        b1_all = const.tile([MT, NMT], F32, tag="b1a")
        w2_all = const2.tile([MT, NMT * OUT_PAD], BF16, tag="w2a")
        b2_sb = const2.tile([OUT_PAD, 1], F32, tag="b2")

        def w_sb(t, m):
            return wt_m[m][:, t * MT:(t + 1) * MT]

        # PE p-state warmup: dummy matmuls over a memset tile, no DMA deps,
        # so the PE ramps to full clock while chunk 0 is still in flight.
        warm = const2.tile([KT, 256], BF16, tag="warm")
        nc.vector.memset(warm[:], 0)
        wps = hps.tile([MT, 256], F32, tag="warm_ps", bufs=1)
        for _ in range(N_WARM):
            nc.tensor.matmul(wps[:], warm[:, :MT], warm[:], start=True, stop=True)

        # Output accumulates in SBUF, streamed out in OGRP slabs on the
        # GpSimd (SWDGE) ring so stores overlap compute and never queue
        # behind x prefetch loads (FIFO slot-wait deadlock).
        oall = [
            opool.tile([OUT_PAD, OGRP], F32, tag=f"o{g}", name=f"oall{g}")
            for g in range(n_ogrp)
        ]

        # GEMM2 runs one chunk behind GEMM1 (software pipeline): by the
        # time it streams h, the relu that produced h is long done, so the
        # PE never stalls on the ACT semaphore.
        pending = []  # [(hb, j0, slen)] sub-blocks awaiting GEMM2

        def flush_gemm2():
            while pending:
                hb2, j0, slen = pending.pop(0)
                po = ops.tile([OUT_PAD, slen], F32, tag="po", name="po")
                for m in range(NMT):
                    nc.tensor.matmul(
                        po[:],
                        w2_all[:, m * OUT_PAD:(m + 1) * OUT_PAD],
                        hb2[m][:],
                        start=(m == 0),
                        stop=(m == NMT - 1),
                    )
                g = j0 // OGRP
                nc.vector.tensor_scalar_add(
                    oall[g][:, j0 - g * OGRP:j0 - g * OGRP + slen],
                    po[:], b2_sb[:],
                )
                if (j0 + slen) % OGRP == 0:
                    nc.gpsimd.dma_start(
                        outT[:, g * OGRP:(g + 1) * OGRP], oall[g][:]
                    )

        # --- main loop over batch chunks ---
        coff = 0
        for c, csz in enumerate(chunks):
            xt = []
            for t in range(NKT):
                xtile = xpool.tile([KT, csz], BF16, tag=f"x{t}")
                # chunk 0's odd k-tiles ride the scalar ring (behind the
                # weights) so the first PSUM group is ready sooner.  Later
                # chunks must NOT: scalar-ring transfers queue ahead of the
                # ACTIVATEs and stall the PSUM-bank recycle (measured).
                eng = nc.scalar if (c == 0 and t % 2 == 1) else nc.sync
                eng.dma_start(
                    xtile[:], xT[t * KT:(t + 1) * KT, coff:coff + csz]
                )
                xt.append(xtile)
            if c == 0:
                # small one-time constants, behind chunk 0's x tiles
                nc.scalar.dma_start(b1_all[:], b1a[:, :])
                nc.scalar.dma_start(w2_all[:], w2a[:, :])
                nc.scalar.dma_start(b2_sb[:], b2c[:, :])
            blocks = blocks_of(csz)
            if c < 3:
                # Head chunks run t-major: each x tile feeds all (s, m)
                # PSUM groups as soon as it lands, so the PE rides the DMA
                # arrival rate instead of stalling on each chunk's last
                # k-tile while the prefetch pipeline is still empty.
                # (Steady state m-major streams ~3us faster overall.)
                pss = {
                    (s, m): hps.tile([MT, SUB], F32, tag=f"ps{m}",
                                     name=f"ps_c{c}_s{s}_m{m}")
                    for s in range(len(blocks))
                    for m in range(NMT)
                }
                for t in range(NKT):
                    for s, (soff, slen) in enumerate(blocks):
                        for m in range(NMT):
                            nc.tensor.matmul(
                                pss[s, m][:, :slen],
                                w_sb(t, m),
                                xt[t][:, soff:soff + slen],
                                start=(t == 0),
                                stop=(t == NKT - 1),
                            )
                    if t == 3:
                        flush_gemm2()  # prev chunk's GEMM2, relus long done
                for s, (soff, slen) in enumerate(blocks):
                    hb = []
                    for m in range(NMT):
                        h = hpool.tile([MT, slen], BF16, tag=f"h{m}")
                        nc.scalar.activation(h[:], pss[s, m][:, :slen], relu,
                                             bias=b1_all[:, m:m + 1])
                        hb.append(h)
                    pending.append((hb, coff + soff, slen))
            else:
                for s, (soff, slen) in enumerate(blocks):
                    hb = []
                    for m in range(NMT):
                        ps = hps.tile([MT, SUB], F32, tag=f"ps{m}")
                        for t in range(NKT):
                            nc.tensor.matmul(
                                ps[:, :slen],
                                w_sb(t, m),
                                xt[t][:, soff:soff + slen],
                                start=(t == 0),
                                stop=(t == NKT - 1),
                            )
                        h = hpool.tile([MT, slen], BF16, tag=f"h{m}")
                        nc.scalar.activation(h[:], ps[:, :slen], relu,
                                             bias=b1_all[:, m:m + 1])
                        hb.append(h)
                        if s == 0 and m == 0:
                            flush_gemm2()  # prev chunk's GEMM2, relus ready
                    pending.append((hb, coff + soff, slen))
            coff += csz
        flush_gemm2()

    nc.compile()
    return nc


def _get_nc(b_shard: int = B_SHARD):
    if b_shard not in _CACHE:
        _CACHE[b_shard] = _build(b_shard)
    return _CACHE[b_shard]


def _host_prep(x, w_conv, w1, b1, w2, b2, b_shard=B_SHARD):
    """Fold conv into w1, pack weights, and lay out per-core inputs."""
    bf16 = ml_dtypes.bfloat16
    # Conv matrix Wc[784, 676]: feat[:, oi*26+oj] = sum_{di,dj} x[:, (oi+di)*28+(oj+dj)] * w_conv[di,dj]
    w_conv = np.asarray(w_conv, np.float64)
    oi = np.arange(26)
    oj = np.arange(26)
    wc = np.zeros((784, 676), np.float64)
    for di in range(3):
        for dj in range(3):
            src = ((oi[:, None] + di) * 28 + (oj[None, :] + dj)).ravel()
            dst = (oi[:, None] * 26 + oj[None, :]).ravel()
            wc[src, dst] += w_conv[di, dj]
    w_eff = (wc @ np.asarray(w1, np.float64)).astype(bf16)  # [784, 256]

    # wta[p, (m*NKT+t)*MT + j] = w_eff[t*KT+p, m*MT+j]  (m-major)
    wta = np.ascontiguousarray(
        w_eff.reshape(NKT, KT, NMT, MT).transpose(1, 2, 0, 3).reshape(KT, -1)
    )
    # b1a[p, m] = b1[m*MT+p]
    b1a = np.ascontiguousarray(
        np.asarray(b1, np.float32).reshape(NMT, MT).T
    )
    # w2a[p, m*OUT_PAD + j] = w2_padded[m*MT+p, j]
    w2p = np.zeros((CH, OUT_PAD), bf16)
    w2p[:, :OUT_CH] = np.asarray(w2).astype(bf16)
    w2a = np.ascontiguousarray(
        w2p.reshape(NMT, MT, OUT_PAD).transpose(1, 0, 2).reshape(MT, -1)
    )
    b2c = np.zeros((OUT_PAD, 1), np.float32)
    b2c[:OUT_CH, 0] = np.asarray(b2, np.float32)

    x_bf = np.asarray(x).astype(bf16)  # [B, 784]
    in_maps = []
    for c in range(N_CORES):
        shard = x_bf[c * b_shard:(c + 1) * b_shard]
        in_maps.append(
            {
                "xT": np.ascontiguousarray(shard.T),  # [784, b_shard]
                "wta": wta,
                "b1a": b1a,
                "w2a": w2a,
                "b2c": b2c,
            }
        )
    return in_maps


LAST_RESULT = None  # BassKernelResults of the most recent run (for test harness)


def kernel(x, w_conv, w1, b1, w2, b2):
    global LAST_RESULT
    nc = _get_nc()
    in_maps = _host_prep(x, w_conv, w1, b1, w2, b2)
    trace = bool(int(os.environ.get("KERNEL_TRACE", "0")))
    res = run_bass_kernel_spmd(
        nc, in_maps, list(range(N_CORES)), trace=trace,
        tmpdir=os.environ.get("KERNEL_TMPDIR") or None,
    )
    LAST_RESULT = res
    out = np.empty((B, OUT_CH), np.float32)
    for c in range(N_CORES):
        out[c * B_SHARD:(c + 1) * B_SHARD] = res.results[c]["outT"][:OUT_CH].T
    return out
